# revision 1
# baseline (speedup 1.0000x reference)
"""Trainium2 Bass kernel for nn_Attention_13048110645532.

Computes, for B=64, S=2048, H=1024 (fp32):
    energy = tanh(hidden @ Wh + encoder_outputs @ We + b_attn)   # [B, S, H]
    scores = energy @ v                                          # [B, S]
    scores = where(mask == 0, -1e9, scores)
    out    = softmax(scores, axis=1)                             # [B, S]

Strategy: data-parallel over batch across 8 NeuronCores (8 batches/core),
attn/v weights replicated.

Mask sparsity: softmax(where(mask==0, -1e9, s)) is exactly 0 at masked
positions, so only unmasked rows are computed. All of a core's unmasked
(batch, s) positions are packed into one stream of 128-row windows
(cross-batch packing: ~65 windows/core vs 80 for per-batch padding).

All matmul operands are bfloat16 (rel err ~1.4e-3 vs the 2e-2 gate; the
host casts encoder_outputs/weights once). bf16 runs at the full PE rate
(1 col/cycle) like f32r, but additionally:
  - dma_gather(transpose=True) transposes 2-byte rows during the gather,
    so X^T (k on partitions) materializes straight from HBM -- no PE
    transpose passes and no PSUM->SBUF copy traffic at all;
  - HBM traffic for the big tensor halves.

Batches are dealt to cores by descending unmasked count and each batch
gets a core-invariant slot range [P[b], P[b+1]) (width = max count over
cores), so the batch->slot boundaries are compile-time constants shared
by the single SPMD program (~65 windows/core vs 80 for per-batch
padding).

Energy is computed transposed (h on partitions, s on free dim): We tiles
are stationary operands in their native layout; the per-position bias
(hidden @ Wh + b_attn)[batch_of(s)], constant on each compile-time batch
run, rides the tanh ACT as a per-partition bias column. The v-dot runs
off the PE: DVE scalar_tensor_tensor accumulates v_m * tanh_m across
h-tiles, a Pool partition_all_reduce finishes the h-sum, and ACT exps
the score row. Raw exp values stream to HBM per chunk (bf16); the host
sums each batch's valid slice and normalizes during the scatter.

Chunk 0 runs k-major in two 4-h-tile passes (one accumulation group per
PSUM bank) so the PE consumes each We tile as its DMA lands. A chain of
dependency-free warmup matmuls holds the PE from t~0 so the p-state
ramp (0.65->2.4GHz after 3us continuous busy) completes before real
work; the gathers' Pool-side descriptor-gen is gated by splitting the
idx DMA so they cannot cut the DMA FIFO ahead of the weight tiles. The
tiny bias tensor hb = hidden @ Wh + b_attn (0.02% of FLOPs) comes from
the host in f32.

The masked softmax needs no max-subtraction: |scores| <= sum|v| (~16,
exp safely in fp32 range); padded slots never reach the output (the
host scatter reads only each batch's valid slice).
"""

import os
import sys
from contextlib import ExitStack

import numpy as np

for _p in ("/opt/trn_rl_repo", os.path.expanduser("~/.axon_site/_ro/trn_rl_repo")):
    if os.path.isdir(_p) and _p not in sys.path:
        sys.path.insert(0, _p)

N_CORES = 8
B, S, H = 64, 2048, 1024
CW = 4  # windows per matmul chunk (SC = CW*128 moving columns, one PSUM bank)


def _chunks(NWIN):
    """Chunk layout [(first_window, n_windows)]: 2-window chunks 0 and 1
    (chunk 0 shortens the startup critical path; chunk 1's half-size gather
    lands before the PE drains chunk 0), CW-window chunks, and a 1-window
    final chunk (shorter tanh->vdot->exp->store tail)."""
    out = []
    w = 0
    while w < NWIN:
        left = NWIN - w
        if w in (0, 2):
            cw = min(2, left)
        elif left <= CW and left > 1:
            cw = left - 1
        else:
            cw = min(CW, left)
        out.append((w, cw))
        w += cw
    return out


def emit(ctx, tc, io, BPC, S, H, NWIN, runs, bufs=None):
    import concourse.bass as bass  # noqa: F401
    from concourse import mybir

    nc = tc.nc
    f32 = mybir.dt.float32
    bf16 = mybir.dt.bfloat16
    TANH = mybir.ActivationFunctionType.Tanh
    EXP = mybir.ActivationFunctionType.Exp

    K2 = 2 * H  # contraction size of the encoder matmul
    KT = K2 // 128  # k-tiles of the encoder matmul
    HT = H // 128  # h-tiles
    HD = H // 128  # k-tiles of the hidden@Wh matmul
    NTOTP = NWIN * 128
    chunks = _chunks(NWIN)

    hbt_d, enc_d, idx_d, web_d, vr_d, out_d, acl_d = io
    enc_flat = enc_d.rearrange("b s k -> (b s) k")

    bufs = dict(bufs or {})
    nb = lambda k, d: bufs.get(k, d)
    singles = ctx.enter_context(tc.tile_pool(name="singles", bufs=1))
    xtp = ctx.enter_context(tc.tile_pool(name="xtp", bufs=nb("xtp", 3)))
    tsbp = ctx.enter_context(tc.tile_pool(name="tsbp", bufs=nb("tsbp", 4)))
    accp = ctx.enter_context(tc.tile_pool(name="accp", bufs=nb("accp", 2)))
    scp = ctx.enter_context(tc.tile_pool(name="scp", bufs=nb("scp", 2)))
    epp = ctx.enter_context(tc.tile_pool(name="epp", bufs=nb("epp", 5), space="PSUM"))
    tpp = ctx.enter_context(tc.tile_pool(name="tpp", bufs=1, space="PSUM"))

    # Gather indices in three DMAs. The gathers' descriptor-gen runs on the
    # Pool engine (parallel to HWDGE), so a gather becomes DMA-eligible the
    # moment its idx columns land -- gating chunk 1+'s idx transfer behind
    # the We tiles in the HWDGE queue keeps those ~6us gathers from cutting
    # ahead of the weights chunk 0 is drip-feeding on.
    idx_sb = singles.tile([128, NWIN * 8], mybir.dt.int16)
    c0w = chunks[0][1] * 8
    c2w = (chunks[2][0] + chunks[2][1]) * 8 if len(chunks) > 2 else NWIN * 8
    nc.sync.dma_start(out=idx_sb[:, :c0w], in_=idx_d[:, :c0w])

    def load_idx_mid():
        if c2w > c0w:
            nc.sync.dma_start(out=idx_sb[:, c0w:c2w], in_=idx_d[:, c0w:c2w])

    def load_idx_rest():
        if NWIN * 8 > c2w:
            nc.sync.dma_start(out=idx_sb[:, c2w:], in_=idx_d[:, c2w:])

    def produce_xt(ci):
        w0, cw = chunks[ci]
        # X^T for one chunk via transposed gather(s). Chunks 1-2 gather
        # per-window so their transfers interleave with the weight tiles on
        # the serialized DMA engine instead of blocking them for ~6us
        # (chunk 0 drip-feeds on those weights); later chunks use one fused
        # gather (weights are resident by then).
        xt = xtp.tile([128, KT, cw * 128], bf16, name="xt")
        nc.gpsimd.dma_gather(
            out_ap=xt,
            in_ap=enc_flat,
            idxs_ap=idx_sb[:, w0 * 8 : (w0 + cw) * 8],
            num_idxs=cw * 128,
            num_idxs_reg=cw * 128,
            elem_size=K2,
            transpose=True,
        )
        return ("f", xt)

    # Dependency-free warmup matmuls: hold the PE busy from ~t=0 so its
    # p-state ramp (0.65 -> 2.4 GHz after 3us of continuous work) completes
    # before chunk 0's real matmuls start; results are never read.
    warm_sb = singles.tile([1, 512], bf16)
    nc.gpsimd.memset(warm_sb, 0.0)
    warm_ps = tpp.tile([1, 512], f32, tag="tp", name="warm")
    for _ in range(20):
        nc.tensor.matmul(
            warm_ps, warm_sb[:, :1], warm_sb, start=True, stop=True
        )

    cur = produce_xt(0)

    # Per-position tanh bias (hidden @ Wh + b_attn, host-computed: 0.02% of
    # the FLOPs), transposed [128(h), HT, BPC] -- a 256B/partition DMA, so
    # the bias is resident long before the first tanh. v likewise.
    hbT = singles.tile([128, HT, BPC], f32)
    nc.sync.dma_start(out=hbT, in_=hbt_d.rearrange("(t p) b -> p t b", p=128))
    v_sb = singles.tile([128, HT], f32)
    nc.sync.dma_start(out=v_sb, in_=vr_d.rearrange("(t p) -> p t", p=128))

    # We resident as KT row-blocks [128, H], k on partitions (native layout),
    # consumed in k order by chunk 0 as the tiles land.
    web_sb = singles.tile([128, KT * H], bf16)

    def load_web(t):
        nc.sync.dma_start(
            out=web_sb.rearrange("p (t h) -> p t h", t=KT)[:, t],
            in_=web_d[t * 128 : (t + 1) * 128, :],
        )

    for t in range(KT):
        load_web(t)
    load_idx_mid()
    load_idx_rest()

    nxt = produce_xt(1) if len(chunks) > 1 else None

    def tanh_acc(ci, m, ep, acc, SC):
        tsb = tsbp.tile([128, SC], bf16, tag="tsb", name="tsb")
        # The per-position bias hb[batch_of(j)] is constant on each batch
        # run of the packed stream (compile-time): per-run ACT bias.
        for cs, ce, b in runs[ci]:
            nc.scalar.activation(
                tsb[:, cs:ce],
                ep[:, cs:ce],
                TANH,
                bias=hbT[:, m, b : b + 1],
                scale=1.0,
            )
        # v-dot rides the DVE: acc += tanh * v_m (per-partition scalar).
        if m == 0:
            nc.vector.tensor_scalar_mul(acc[:, :SC], tsb, v_sb[:, 0:1])
        else:
            nc.vector.scalar_tensor_tensor(
                acc[:, :SC],
                tsb,
                v_sb[:, m : m + 1],
                acc[:, :SC],
                op0=mybir.AluOpType.mult,
                op1=mybir.AluOpType.add,
            )

    def energy_mm(ep, m, k, xt, SC):
        kind, tile = xt
        rhs = tile[:, :, k, :] if kind == "w" else tile[:, k, :]
        nc.tensor.matmul(
            ep[:, :SC],
            web_sb[:, k * H + m * 128 : k * H + (m + 1) * 128],
            rhs,
            start=(k == 0),
            stop=(k == KT - 1),
        )

    def mm_chunk(ci, xt):
        w0, cw = chunks[ci]
        SC = cw * 128
        sl = slice(w0 * 128, w0 * 128 + SC)
        acc = accp.tile([128, 512], f32, name="acc")

        if ci == 0:
            # Chunk 0 runs k-major in two 4-m passes on 4 full PSUM banks
            # (one accumulation group per bank): pass A consumes each We tile
            # the moment its DMA lands, the hb chain slots between passes,
            # pass B runs on resident weights.
            for half in range(2):
                eps = [
                    epp.tile([128, 512], f32, tag="ep", name=f"ep{half}{i}")
                    for i in range(4)
                ]
                for k in range(KT):
                    for i in range(4):
                        energy_mm(eps[i], half * 4 + i, k, xt, SC)
                for i in range(4):
                    tanh_acc(ci, half * 4 + i, eps[i], acc, SC)
        else:
            for m in range(HT):
                ep = epp.tile([128, 512], f32, tag="ep", name="ep")
                for k in range(KT):
                    energy_mm(ep, m, k, xt, SC)
                tanh_acc(ci, m, ep, acc, SC)
        if ci == len(chunks) - 1 and SC == 128:
            # Final chunk: ship the pre-reduce v-dot accumulator and let the
            # host finish sum+exp for these 128 columns -- drops the
            # allreduce->exp links from the terminal dependency chain.
            nc.sync.dma_start(out=acl_d, in_=acc[:, :SC])
            return
        # Partition-all-reduce the v-weighted tanh (Pool): every partition
        # gets the score row; the batch rows 0..BPC-1 feed the masked exp.
        import concourse.bass_isa as bass_isa

        scB = scp.tile([128, 512], f32, tag="scB", name="scB")
        nc.gpsimd.partition_all_reduce(
            scB[:, :SC], acc[:, :SC], channels=128,
            reduce_op=bass_isa.ReduceOp.add,
        )
        # Stream raw exp(scores) straight to HBM (bf16) as each chunk lands;
        # the host sums the valid slice and normalizes during the scatter, so
        # the device tail is just the last chunk's exp + its store.
        esb = tsbp.tile([BPC, SC], bf16, tag="esb", name="esb")
        nc.scalar.activation(esb, scB[:BPC, :SC], EXP)
        nc.sync.dma_start(out=out_d[:, sl], in_=esb)

    # Software-pipelined emission: chunk ci+2's gathers are emitted (= higher
    # Tile priority) before chunk ci's matmuls.
    for ci in range(len(chunks)):
        nxt2 = produce_xt(ci + 2) if ci + 2 < len(chunks) else None
        mm_chunk(ci, cur)
        cur = nxt
        nxt = nxt2


def build_nc(BPC, S, H, NWIN, runs, bufs=None):
    import concourse.tile as tile
    from concourse import bacc, mybir

    f32 = mybir.dt.float32
    bf16 = mybir.dt.bfloat16
    i16 = mybir.dt.int16

    NTOTP = NWIN * 128
    nc = bacc.Bacc("TRN2", target_bir_lowering=False, debug=False)
    hbt_d = nc.dram_tensor("hbt", [H, BPC], f32, kind="ExternalInput").ap()
    enc_d = nc.dram_tensor("enc", [BPC, S, 2 * H], bf16, kind="ExternalInput").ap()
    idx_d = nc.dram_tensor("idxw", [128, NWIN * 8], i16, kind="ExternalInput").ap()
    web_d = nc.dram_tensor("web", [2 * H, H], bf16, kind="ExternalInput").ap()
    vr_d = nc.dram_tensor("vrep", [H], f32, kind="ExternalInput").ap()
    out_d = nc.dram_tensor("out", [BPC, NTOTP], bf16, kind="ExternalOutput").ap()
    acl_d = nc.dram_tensor("accl", [128, 128], f32, kind="ExternalOutput").ap()
    io = (hbt_d, enc_d, idx_d, web_d, vr_d, out_d, acl_d)

    with tile.TileContext(nc) as tc:
        with ExitStack() as ctx:
            emit(ctx, tc, io, BPC, S, H, NWIN, runs, bufs=bufs)
    nc.compile()
    return nc


_NC_CACHE = {}


def _get_nc(BPC, S, H, NWIN, runs):
    key = (BPC, S, H, NWIN, runs)
    if key not in _NC_CACHE:
        _NC_CACHE[key] = build_nc(BPC, S, H, NWIN, runs)
    return _NC_CACHE[key]


def _chunk_runs(NWIN, P):
    """Per-chunk (colstart, colend, batch) runs from the uniform segment
    boundaries P (len BPC+1); the tail after P[-1] rides with the last batch
    (its tanh output is finite garbage, zeroed by the indicator)."""
    NTOTP = NWIN * 128
    BPC = len(P) - 1
    segs = [(P[b], P[b + 1], b) for b in range(BPC) if P[b + 1] > P[b]]
    if not segs:
        segs = [(0, NTOTP, 0)]
    s0, _, b0 = segs[-1]
    segs[-1] = (s0, NTOTP, b0)
    runs = []
    for w0, cw in _chunks(NWIN):
        c0, c1 = w0 * 128, (w0 + cw) * 128
        rr = []
        for s, e, b in segs:
            lo, hi = max(s, c0), min(e, c1)
            if lo < hi:
                rr.append((lo - c0, hi - c0, b))
        if not rr:
            rr.append((0, c1 - c0, segs[-1][2]))
        # cover any gap at the chunk head (before the first segment)
        if rr[0][0] != 0:
            rr.insert(0, (0, rr[0][0], rr[0][2]))
        runs.append(tuple(rr))
    return tuple(runs)


def _pack_meta(mask, BPC, S):
    """Uniform segmented packing: batch b occupies slots [P[b], P[b+1]) on
    every core (P from per-batch max counts over cores), so the batch->slot
    boundaries are core-invariant compile-time constants. Returns per-core
    wrapped int16 gather indices, batch-indicator matrices, NWIN, P."""
    n_cores = mask.shape[0] // BPC
    m3 = mask.astype(bool).reshape(n_cores, BPC, S)
    cnt = m3.sum(axis=2)  # [n_cores, BPC]
    seg = cnt.max(axis=0)  # [BPC]
    P = np.concatenate([[0], np.cumsum(seg)]).astype(np.int64)
    NWIN = max(2, int(-(-P[-1] // 128)))
    NTOTP = NWIN * 128
    idxw = np.zeros((n_cores, 128, NWIN * 8), dtype=np.int16)
    for core in range(n_cores):
        g = np.zeros((NTOTP,), dtype=np.int64)
        for b in range(BPC):
            s_idx = np.nonzero(m3[core, b])[0]
            n = len(s_idx)
            g[P[b] : P[b] + n] = b * S + s_idx
        # wrapped layout: element (p, w*8 + c) = g[w*128 + c*16 + p],
        # replicated across the 8 Q7 cores' 16-partition groups.
        gw = g.reshape(NWIN, 8, 16).transpose(2, 0, 1)  # [16, NWIN, 8]
        idxw[core] = np.tile(gw.reshape(16, NWIN * 8), (8, 1))
    return idxw, NWIN, tuple(int(x) for x in P)


def kernel(hidden, encoder_outputs, mask, W_attn, b_attn, v):
    import ml_dtypes

    from concourse.bass_utils import run_bass_kernel_spmd

    bf16 = ml_dtypes.bfloat16
    hidden = np.asarray(hidden, dtype=np.float32)
    mask = np.asarray(mask, dtype=np.int32)
    W_attn = np.asarray(W_attn, dtype=np.float32)

    B_, S_ = mask.shape
    H_ = hidden.shape[1]
    BPC = B_ // N_CORES

    web = np.ascontiguousarray(W_attn[H_:].astype(bf16))
    # hidden @ Wh + b_attn: 0.02% of the FLOPs, computed host-side in f32.
    hb = hidden @ W_attn[:H_] + np.asarray(b_attn, dtype=np.float32)
    vrep = np.ascontiguousarray(np.asarray(v, dtype=np.float32))

    # Deal batches to cores by descending unmasked count (rank r -> core r%8,
    # slot r//8): slot-mates have near-equal counts, minimizing the padded
    # segment sizes (slot width = max over cores) of the uniform packing.
    counts = mask.astype(bool).sum(axis=1)
    order = np.argsort(-counts, kind="stable")
    perm = np.empty_like(order)  # perm[core*BPC + slot] = global batch
    for r, gb in enumerate(order):
        perm[(r % N_CORES) * BPC + r // N_CORES] = gb

    maskp = mask[perm]
    idxw, NWIN, P = _pack_meta(maskp, BPC, S_)
    runs = _chunk_runs(NWIN, P)

    enc = np.asarray(encoder_outputs)
    nc = _get_nc(BPC, S_, H_, NWIN, runs)
    in_maps = [
        {
            "hbt": np.ascontiguousarray(hb[perm[i * BPC : (i + 1) * BPC]].T),
            "enc": enc[perm[i * BPC : (i + 1) * BPC]].astype(bf16),
            "idxw": idxw[i],
            "web": web,
            "vrep": vrep,
        }
        for i in range(N_CORES)
    ]
    res = run_bass_kernel_spmd(nc, in_maps, list(range(N_CORES)))

    out = np.zeros((B_, S_), dtype=np.float32)
    last1 = _chunks(NWIN)[-1][1] == 1
    for core in range(N_CORES):
        packed = np.asarray(res.results[core]["out"], dtype=np.float32)
        if last1:
            accl = np.asarray(res.results[core]["accl"], dtype=np.float32)
            packed[:, -128:] = np.exp(accl.sum(axis=0))[None, :]
        for b in range(BPC):
            gb = perm[core * BPC + b]
            s_idx = np.nonzero(mask[gb])[0]
            if len(s_idx):
                e = packed[b, P[b] : P[b] + len(s_idx)]
                out[gb, s_idx] = e / e.sum(dtype=np.float64)
    allmasked = ~mask.astype(bool).any(axis=1)
    if allmasked.any():
        # Reference softmaxes a constant -1e9 row: exactly uniform.
        out[allmasked] = np.float32(1.0) / np.float32(S_)
    return out



# revision 3
# speedup vs baseline: 1.9157x; 1.9157x over previous
"""Trainium2 Bass kernel for nn_Attention_13048110645532.

Computes, for B=64, S=2048, H=1024 (fp32):
    energy = tanh(hidden @ Wh + encoder_outputs @ We + b_attn)   # [B, S, H]
    scores = energy @ v                                          # [B, S]
    scores = where(mask == 0, -1e9, scores)
    out    = softmax(scores, axis=1)                             # [B, S]

Strategy: data-parallel over batch across 8 NeuronCores (8 batches/core),
attn/v weights replicated.

Mask sparsity: softmax(where(mask==0, -1e9, s)) is exactly 0 at masked
positions, so only unmasked rows are computed. All of a core's unmasked
(batch, s) positions are packed into one stream of 128-row windows
(cross-batch packing: ~65 windows/core vs 80 for per-batch padding).

The matmul runs in fp8(e4m3) DoubleRow perf mode: each matmul contracts
TWO 128-k-tiles at 0.5 cycles per output column -- 4x the bf16 rate.
Plain e4m3 quantization of X=encoder_outputs and We fails the 2e-2 gate
(rel err 2.7e-2), so X is split hi+lo: X ~ (X8 + XL)/sx with
X8 = e4m3(sx*X), XL = e4m3(sx*X - X8), both at the SAME scale so the two
products accumulate directly in one f32 PSUM group. The device sees an
augmented contraction: XT_aug = [X8^T; XL^T] in HBM [4096, packed_S],
with the 16 e4m3 We k-tiles reused for both halves (k mod 16). Measured
end-to-end rel err 1.3e-2 vs the 2e-2 gate; PE time 2x bf16 rate.

The host does the gather/transpose/quantization (masked packing straight
into XT_aug), so the device streams plain contiguous tiles -- no gather,
no index upload, no gpsimd descriptor generation.

Energy is computed transposed (h on partitions, s on free dim): We tiles
are stationary operands in their native layout; the per-position bias
(hidden @ Wh + b_attn)[batch_of(s)], constant on each compile-time batch
run, rides the tanh ACT as a per-partition bias column together with the
fp8 dequant scale 1/(sx*sw). The v-dot runs off the PE: DVE
scalar_tensor_tensor accumulates v_m * tanh_m across h-tiles, a Pool
partition_all_reduce finishes the h-sum, and ACT exps the score row. Raw
exp values stream to HBM per chunk (f32); the host sums each batch's
valid slice and normalizes during the scatter.

Chunk 0 runs k-major in two 4-h-tile passes (one accumulation group per
PSUM bank) so the PE consumes each We-pair/X-pair as its DMA lands. A
chain of dependency-free warmup matmuls holds the PE from t~0 so the
p-state ramp (0.65->2.4GHz after 3us continuous busy) completes before
real work. The tiny bias tensor hb = hidden @ Wh + b_attn (0.02% of
FLOPs) comes from the host in f32.

The masked softmax needs no max-subtraction: |scores| <= sum|v| (~16,
exp safely in fp32 range); padded slots never reach the output (the
host scatter reads only each batch's valid slice).
"""

import os
import sys
from contextlib import ExitStack

import numpy as np

for _p in ("/opt/trn_rl_repo", os.path.expanduser("~/.axon_site/_ro/trn_rl_repo")):
    if os.path.isdir(_p) and _p not in sys.path:
        sys.path.insert(0, _p)

N_CORES = 8
B, S, H = 64, 2048, 1024
CW = 4  # windows per matmul chunk (SC = CW*128 moving columns, one PSUM bank)
SX = 16.0  # e4m3 scale for X (enc); max |enc| ~5.4 -> 87 < 240
SW = 4096.0  # e4m3 scale for We; max |We| ~0.018 -> 74 < 240


def _chunks(NWIN):
    """Chunk layout [(first_window, n_windows)]: CW-window chunks and a
    1-window final chunk (shorter tanh->vdot->exp->store tail; its v-dot
    accumulator finishes on the host)."""
    out = []
    w = 0
    while w < NWIN:
        left = NWIN - w
        if left <= CW and left > 1:
            cw = left - 1
        else:
            cw = min(CW, left)
        out.append((w, cw))
        w += cw
    return out


def emit(ctx, tc, io, BPC, S, H, NWIN, runs, bufs=None):
    import concourse.bass as bass  # noqa: F401
    from concourse import mybir

    nc = tc.nc
    f32 = mybir.dt.float32
    bf16 = mybir.dt.bfloat16
    fp8 = mybir.dt.float8e4
    DR = mybir.MatmulPerfMode.DoubleRow
    TANH = mybir.ActivationFunctionType.Tanh
    EXP = mybir.ActivationFunctionType.Exp

    KA = 4 * H  # augmented contraction (hi + lo fp8 planes)
    KT = KA // 128  # 32 augmented k-tiles
    KP = KT // 2  # 16 DoubleRow k-tile pairs
    WKT = 2 * H // 128  # 16 real We k-tiles
    HT = H // 128  # h-tiles
    NTOTP = NWIN * 128
    chunks = _chunks(NWIN)
    DEQ = 1.0 / (SX * SW)

    hbt_d, xt_d, web_d, vr_d, out_d, acl_d = io
    xt_r = xt_d.rearrange("(t p) s -> p t s", p=128)  # [128, KT, NTOTP]

    bufs = dict(bufs or {})
    nb = lambda k, d: bufs.get(k, d)
    singles = ctx.enter_context(tc.tile_pool(name="singles", bufs=1))
    xtp = ctx.enter_context(tc.tile_pool(name="xtp", bufs=nb("xtp", 3)))
    tsbp = ctx.enter_context(tc.tile_pool(name="tsbp", bufs=nb("tsbp", 4)))
    accp = ctx.enter_context(tc.tile_pool(name="accp", bufs=nb("accp", 2)))
    scp = ctx.enter_context(tc.tile_pool(name="scp", bufs=nb("scp", 2)))
    epp = ctx.enter_context(tc.tile_pool(name="epp", bufs=nb("epp", 5), space="PSUM"))
    tpp = ctx.enter_context(tc.tile_pool(name="tpp", bufs=1, space="PSUM"))

    # Dependency-free warmup matmuls: hold the PE busy from ~t=0 so its
    # p-state ramp (0.65 -> 2.4 GHz after 3us of continuous work) completes
    # before chunk 0's real matmuls start; results are never read.
    warm_sb = singles.tile([1, 512], bf16)
    nc.gpsimd.memset(warm_sb, 0.0)
    warm_ps = tpp.tile([1, 512], f32, tag="tp", name="warm")
    for _ in range(20):
        nc.tensor.matmul(
            warm_ps, warm_sb[:, :1], warm_sb, start=True, stop=True
        )

    # Per-position tanh bias (hidden @ Wh + b_attn, host-computed: 0.02% of
    # the FLOPs), transposed [128(h), HT, BPC] -- a 256B/partition DMA, so
    # the bias is resident long before the first tanh. v likewise.
    hbT = singles.tile([128, HT, BPC], f32)
    nc.sync.dma_start(out=hbT, in_=hbt_d.rearrange("(t p) b -> p t b", p=128))
    v_sb = singles.tile([128, HT], f32)
    nc.sync.dma_start(out=v_sb, in_=vr_d.rearrange("(t p) -> p t", p=128))

    # We (e4m3) resident as [128, WKT, H], k on partitions; pair p of the
    # augmented contraction uses We k-tiles (2p mod WKT, 2p+1 mod WKT) --
    # the hi and lo X planes share the same weights.
    web_sb = singles.tile([128, WKT, H], fp8)

    def load_web_pair(t):
        nc.sync.dma_start(
            out=web_sb[:, 2 * t : 2 * t + 2, :],
            in_=web_d[:, 2 * t : 2 * t + 2, :],
        )

    def produce_xt(ci, split=False):
        w0, cw = chunks[ci]
        SC = cw * 128
        sl = slice(w0 * 128, w0 * 128 + SC)
        xt = xtp.tile([128, KT, SC], fp8, name="xt")
        if split:
            # Per-pair DMAs so chunk 0's k-major pass consumes each pair the
            # moment it lands, interleaved with the We pair loads.
            for t in range(KP):
                if t < KP // 2:
                    load_web_pair(t)
                nc.sync.dma_start(
                    out=xt[:, 2 * t : 2 * t + 2, :],
                    in_=xt_r[:, 2 * t : 2 * t + 2, sl],
                )
        else:
            nc.sync.dma_start(out=xt, in_=xt_r[:, :, sl])
        return xt

    cur = produce_xt(0, split=True)
    nxt = produce_xt(1) if len(chunks) > 1 else None

    def tanh_acc(ci, m, ep, acc, SC):
        tsb = tsbp.tile([128, SC], bf16, tag="tsb", name="tsb")
        # The per-position bias hb[batch_of(j)] is constant on each batch
        # run of the packed stream (compile-time): per-run ACT bias. The
        # fp8 dequant scale rides the same ACT.
        for cs, ce, b in runs[ci]:
            nc.scalar.activation(
                tsb[:, cs:ce],
                ep[:, cs:ce],
                TANH,
                bias=hbT[:, m, b : b + 1],
                scale=DEQ,
            )
        # v-dot rides the DVE: acc += tanh * v_m (per-partition scalar).
        if m == 0:
            nc.vector.tensor_scalar_mul(acc[:, :SC], tsb, v_sb[:, 0:1])
        else:
            nc.vector.scalar_tensor_tensor(
                acc[:, :SC],
                tsb,
                v_sb[:, m : m + 1],
                acc[:, :SC],
                op0=mybir.AluOpType.mult,
                op1=mybir.AluOpType.add,
            )

    def energy_mm(ep, m, t, xt, SC):
        # DoubleRow fp8 matmul: contracts augmented k-tiles (2t, 2t+1) in
        # SC/2 cycles; stationary = the matching We pair (shared hi/lo).
        wt = (2 * t) % WKT
        nc.tensor.matmul(
            ep[:, :SC],
            web_sb[:, wt : wt + 2, m * 128 : (m + 1) * 128],
            xt[:, 2 * t : 2 * t + 2, :],
            start=(t == 0),
            stop=(t == KP - 1),
            perf_mode=DR,
        )

    def mm_chunk(ci, xt):
        w0, cw = chunks[ci]
        SC = cw * 128
        sl = slice(w0 * 128, w0 * 128 + SC)
        acc = accp.tile([128, 512], f32, name="acc")

        if ci == 0:
            # Chunk 0 runs k-major in two 4-m passes on 4 full PSUM banks
            # (one accumulation group per bank): pass A consumes each
            # We/X pair the moment its DMA lands, the hb chain slots
            # between passes, pass B runs on resident tiles.
            for half in range(2):
                eps = [
                    epp.tile([128, 512], f32, tag="ep", name=f"ep{half}{i}")
                    for i in range(4)
                ]
                for t in range(KP):
                    for i in range(4):
                        energy_mm(eps[i], half * 4 + i, t, xt, SC)
                for i in range(4):
                    tanh_acc(ci, half * 4 + i, eps[i], acc, SC)
        else:
            for m in range(HT):
                ep = epp.tile([128, 512], f32, tag="ep", name="ep")
                for t in range(KP):
                    energy_mm(ep, m, t, xt, SC)
                tanh_acc(ci, m, ep, acc, SC)
        if ci == len(chunks) - 1 and SC == 128:
            # Final chunk: ship the pre-reduce v-dot accumulator and let the
            # host finish sum+exp for these 128 columns -- drops the
            # allreduce->exp links from the terminal dependency chain.
            nc.sync.dma_start(out=acl_d, in_=acc[:, :SC])
            return
        # Partition-all-reduce the v-weighted tanh (Pool): every partition
        # gets the score row; the batch rows 0..BPC-1 feed the masked exp.
        import concourse.bass_isa as bass_isa

        scB = scp.tile([128, 512], f32, tag="scB", name="scB")
        nc.gpsimd.partition_all_reduce(
            scB[:, :SC], acc[:, :SC], channels=128,
            reduce_op=bass_isa.ReduceOp.add,
        )
        # Stream raw exp(scores) straight to HBM (f32) as each chunk lands;
        # the host sums the valid slice and normalizes during the scatter, so
        # the device tail is just the last chunk's exp + its store.
        esb = tsbp.tile([BPC, SC], f32, tag="esb", name="esb")
        nc.scalar.activation(esb, scB[:BPC, :SC], EXP)
        nc.sync.dma_start(out=out_d[:, sl], in_=esb)

    # Software-pipelined emission: chunk ci+2's loads are emitted (= higher
    # Tile priority) before chunk ci's matmuls.
    for ci in range(len(chunks)):
        nxt2 = produce_xt(ci + 2) if ci + 2 < len(chunks) else None
        mm_chunk(ci, cur)
        cur = nxt
        nxt = nxt2


def build_nc(BPC, S, H, NWIN, runs, bufs=None):
    import concourse.tile as tile
    from concourse import bacc, mybir

    f32 = mybir.dt.float32
    fp8 = mybir.dt.float8e4

    NTOTP = NWIN * 128
    nc = bacc.Bacc("TRN2", target_bir_lowering=False, debug=False)
    hbt_d = nc.dram_tensor("hbt", [H, BPC], f32, kind="ExternalInput").ap()
    xt_d = nc.dram_tensor("xt", [4 * H, NTOTP], fp8, kind="ExternalInput").ap()
    web_d = nc.dram_tensor("web", [128, 2 * H // 128, H], fp8, kind="ExternalInput").ap()
    vr_d = nc.dram_tensor("vrep", [H], f32, kind="ExternalInput").ap()
    out_d = nc.dram_tensor("out", [BPC, NTOTP], f32, kind="ExternalOutput").ap()
    acl_d = nc.dram_tensor("accl", [128, 128], f32, kind="ExternalOutput").ap()
    io = (hbt_d, xt_d, web_d, vr_d, out_d, acl_d)

    with tile.TileContext(nc) as tc:
        with ExitStack() as ctx:
            emit(ctx, tc, io, BPC, S, H, NWIN, runs, bufs=bufs)
    nc.compile()
    return nc


_NC_CACHE = {}


def _get_nc(BPC, S, H, NWIN, runs):
    key = (BPC, S, H, NWIN, runs)
    if key not in _NC_CACHE:
        _NC_CACHE[key] = build_nc(BPC, S, H, NWIN, runs)
    return _NC_CACHE[key]


def _chunk_runs(NWIN, P):
    """Per-chunk (colstart, colend, batch) runs from the uniform segment
    boundaries P (len BPC+1); the tail after P[-1] rides with the last batch
    (its tanh output is finite garbage, ignored by the host scatter)."""
    NTOTP = NWIN * 128
    BPC = len(P) - 1
    segs = [(P[b], P[b + 1], b) for b in range(BPC) if P[b + 1] > P[b]]
    if not segs:
        segs = [(0, NTOTP, 0)]
    s0, _, b0 = segs[-1]
    segs[-1] = (s0, NTOTP, b0)
    runs = []
    for w0, cw in _chunks(NWIN):
        c0, c1 = w0 * 128, (w0 + cw) * 128
        rr = []
        for s, e, b in segs:
            lo, hi = max(s, c0), min(e, c1)
            if lo < hi:
                rr.append((lo - c0, hi - c0, b))
        if not rr:
            rr.append((0, c1 - c0, segs[-1][2]))
        # cover any gap at the chunk head (before the first segment)
        if rr[0][0] != 0:
            rr.insert(0, (0, rr[0][0], rr[0][2]))
        runs.append(tuple(rr))
    return tuple(runs)


def _pack_meta(mask, BPC, S):
    """Uniform segmented packing: batch b occupies slots [P[b], P[b+1]) on
    every core (P from per-batch max counts over cores), so the batch->slot
    boundaries are core-invariant compile-time constants. Returns per-core
    packed gather row indices (into the core's [BPC*S] row space), NWIN, P."""
    n_cores = mask.shape[0] // BPC
    m3 = mask.astype(bool).reshape(n_cores, BPC, S)
    cnt = m3.sum(axis=2)  # [n_cores, BPC]
    seg = cnt.max(axis=0)  # [BPC]
    P = np.concatenate([[0], np.cumsum(seg)]).astype(np.int64)
    NWIN = max(2, int(-(-P[-1] // 128)))
    NTOTP = NWIN * 128
    gidx = np.zeros((n_cores, NTOTP), dtype=np.int64)
    for core in range(n_cores):
        for b in range(BPC):
            s_idx = np.nonzero(m3[core, b])[0]
            n = len(s_idx)
            gidx[core, P[b] : P[b] + n] = b * S + s_idx
    return gidx, NWIN, tuple(int(x) for x in P)


def kernel(hidden, encoder_outputs, mask, W_attn, b_attn, v):
    import ml_dtypes

    from concourse.bass_utils import run_bass_kernel_spmd

    e4 = ml_dtypes.float8_e4m3
    hidden = np.asarray(hidden, dtype=np.float32)
    mask = np.asarray(mask, dtype=np.int32)
    W_attn = np.asarray(W_attn, dtype=np.float32)

    B_, S_ = mask.shape
    H_ = hidden.shape[1]
    BPC = B_ // N_CORES

    # We quantized e4m3 once, tiled [128(k mod), WKT, H].
    We = W_attn[H_:]
    web = np.ascontiguousarray(
        (We.reshape(2 * H_ // 128, 128, H_) * SW).transpose(1, 0, 2).astype(e4)
    )
    # hidden @ Wh + b_attn: 0.02% of the FLOPs, computed host-side in f32.
    hb = hidden @ W_attn[:H_] + np.asarray(b_attn, dtype=np.float32)
    vrep = np.ascontiguousarray(np.asarray(v, dtype=np.float32))

    # Deal batches to cores by descending unmasked count (rank r -> core r%8,
    # slot r//8): slot-mates have near-equal counts, minimizing the padded
    # segment sizes (slot width = max over cores) of the uniform packing.
    counts = mask.astype(bool).sum(axis=1)
    order = np.argsort(-counts, kind="stable")
    perm = np.empty_like(order)  # perm[core*BPC + slot] = global batch
    for r, gb in enumerate(order):
        perm[(r % N_CORES) * BPC + r // N_CORES] = gb

    maskp = mask[perm]
    gidx, NWIN, P = _pack_meta(maskp, BPC, S_)
    runs = _chunk_runs(NWIN, P)
    NTOTP = NWIN * 128

    enc = np.asarray(encoder_outputs, dtype=np.float32)
    nc = _get_nc(BPC, S_, H_, NWIN, runs)
    in_maps = []
    for i in range(N_CORES):
        encp = enc[perm[i * BPC : (i + 1) * BPC]].reshape(BPC * S_, 2 * H_)
        sel = encp[gidx[i]] * SX  # [NTOTP, 2H] scaled f32
        X8 = sel.astype(e4)
        XL = (sel - X8.astype(np.float32)).astype(e4)
        xt = np.empty((4 * H_, NTOTP), dtype=e4)
        xt[: 2 * H_] = X8.T
        xt[2 * H_ :] = XL.T
        in_maps.append(
            {
                "hbt": np.ascontiguousarray(hb[perm[i * BPC : (i + 1) * BPC]].T),
                "xt": xt,
                "web": web,
                "vrep": vrep,
            }
        )
    res = run_bass_kernel_spmd(nc, in_maps, list(range(N_CORES)))

    out = np.zeros((B_, S_), dtype=np.float32)
    last1 = _chunks(NWIN)[-1][1] == 1
    for core in range(N_CORES):
        packed = np.array(res.results[core]["out"], dtype=np.float32)
        if last1:
            accl = np.asarray(res.results[core]["accl"], dtype=np.float32)
            packed[:, -128:] = np.exp(accl.sum(axis=0))[None, :]
        for b in range(BPC):
            gb = perm[core * BPC + b]
            s_idx = np.nonzero(mask[gb])[0]
            if len(s_idx):
                e = packed[b, P[b] : P[b] + len(s_idx)]
                out[gb, s_idx] = e / e.sum(dtype=np.float64)
    allmasked = ~mask.astype(bool).any(axis=1)
    if allmasked.any():
        # Reference softmaxes a constant -1e9 row: exactly uniform.
        out[allmasked] = np.float32(1.0) / np.float32(S_)
    return out
